# revision 9
# baseline (speedup 1.0000x reference)
"""DFlashAttention (paged KV cache decode-attention block) on 8 Trainium2
NeuronCores.

Sharding: tensor-parallel over heads. Each core owns HQ/8 = 4 query heads and
HK/8 = 1 KV head (GQA group). Wq/Wk/Wv row-sharded, Wo column-sharded; each
core produces a partial output [B*S, HID] which is reduced on the host.

Device kernel layout choices:
  - All big matmuls in float32r (full PE rate, ~1e-4 relative rounding).
  - Projections produce q/k/v in [D, token] layout (head dim on partitions).
  - Scores are computed transposed: [l_chunk(128), (head, s)=512] with the
    KV-cache chunk as the stationary operand, so softmax-sum reduces over
    partitions via a ones-matmul and PV consumes probs directly - no
    transposes anywhere in the attention inner loop.
  - Per-batch cache lengths are baked into the instruction stream at build
    time (kernel() sees cache_seqlens on the host); the final partial chunk
    is masked by writing zeros (ACT copy with scale=0) to the invalid tail
    partitions of the prob tile.
  - RMSNorm per-token scales are folded in as column broadcasts via rank-1
    ones-matmuls; RoPE uses two half-swap DMAs per head plus
    scalar_tensor_tensor ops (keeps every DVE op lane-aligned).
"""

import sys

sys.path.insert(0, "/opt/trn_rl_repo")

import math

import numpy as np

B, S, HID = 4, 128, 4096
D, HQ, HK = 128, 32, 8
PAGES, PSIZE, NPP = 64, 256, 16
THETA = 10000.0
EPS = 1e-6
N_CORES = 8
HQC = HQ // N_CORES  # 4 query heads per core
EC = HQC * D         # 512 output-proj contraction per core
BS = B * S           # 512 tokens
NDCH = HID // 128    # 32 contraction chunks for projections

_CACHE = {}


def _prep_host(x, Wq, Wk, Wv, Wo, q_norm_w, k_norm_w, k_cache, v_cache,
               block_table, cache_seqlens):
    f32 = np.float32
    xT = np.ascontiguousarray(np.asarray(x, f32).reshape(BS, HID).T)

    lens = [int(v) for v in np.asarray(cache_seqlens)]
    pads = [(l + 127) // 128 * 128 for l in lens]
    offs = [0] * B
    for b in range(1, B):
        offs[b] = offs[b - 1] + pads[b - 1]
    total = max(sum(pads), 128)

    bt = np.asarray(block_table)
    kg = np.asarray(k_cache, f32)[bt].reshape(B, NPP * PSIZE, HK, D)
    vg = np.asarray(v_cache, f32)[bt].reshape(B, NPP * PSIZE, HK, D)

    # RoPE angles, range-reduced to [-pi, pi) on the host (index arithmetic
    # only; sin/cos evaluated on device). Mimic the reference's fp32 freqs.
    pos = np.asarray(cache_seqlens, np.float64)[:, None] + np.arange(S)[None, :]
    inv = 1.0 / (THETA ** (np.arange(0, D, 2, dtype=np.float64) / D))
    freqs32 = (pos.astype(f32)[:, :, None] * inv.astype(f32)[None, None, :]).astype(f32)
    fr = np.float64(freqs32)
    two_pi = 2 * np.pi

    def red(a):
        m = np.mod(a, two_pi)
        m = np.where(m >= np.pi, m - two_pi, m)
        return m.astype(f32)

    a_sin = red(fr).reshape(BS, 64).T                      # [64, BS]
    a_cos = red(fr + np.pi / 2).reshape(BS, 64).T
    # duplicate across both partition halves -> [128, BS]
    a_sin2 = np.ascontiguousarray(np.concatenate([a_sin, a_sin], 0))
    a_cos2 = np.ascontiguousarray(np.concatenate([a_cos, a_cos], 0))

    wq = np.asarray(q_norm_w, f32).reshape(D, 1)
    wk = np.asarray(k_norm_w, f32).reshape(D, 1)
    wqB = np.roll(wq, 64, axis=0).copy()   # partner-half weight
    wkB = np.roll(wk, 64, axis=0).copy()

    ones = np.ones((128, 1), f32)
    onesr = np.ones((1, 128), f32)
    ident = np.eye(128, dtype=f32)

    masks_h = np.zeros((1, 128 * B), f32)
    for b in range(B):
        t = lens[b] - (pads[b] // 128 - 1) * 128 if pads[b] > 0 else 128
        masks_h[0, b * 128 + t:(b + 1) * 128] = 1.0

    Wq_ = np.asarray(Wq, f32)
    Wk_ = np.asarray(Wk, f32)
    Wv_ = np.asarray(Wv, f32)
    Wo_ = np.asarray(Wo, f32)

    in_maps = []
    for c in range(N_CORES):
        wqT = np.ascontiguousarray(Wq_[c * EC:(c + 1) * EC, :].T)     # [HID, 512]
        wkvT = np.ascontiguousarray(
            np.concatenate([Wk_[c * D:(c + 1) * D, :],
                            Wv_[c * D:(c + 1) * D, :]], 0).T)         # [HID, 256]
        woT = np.ascontiguousarray(Wo_[:, c * EC:(c + 1) * EC].T)     # [512, HID]
        kT = np.zeros((128, total), f32)
        vC = np.zeros((total, 128), f32)
        for b in range(B):
            nb, ob = lens[b], offs[b]
            if nb > 0:
                kT[:, ob:ob + nb] = kg[b, :nb, c, :].T
                vC[ob:ob + nb, :] = vg[b, :nb, c, :]
        in_maps.append(dict(
            xT=xT, wqT=wqT, wkvT=wkvT, woT=woT,
            kT=np.ascontiguousarray(kT), vC=vC,
            a_sin=a_sin2, a_cos=a_cos2,
            wqA=wq, wqB=wqB, wkA=wk, wkB=wkB,
            epsq=np.full((1, 1), D * EPS, f32),
            epsk=np.full((1, 1), EPS, f32),
            masks=masks_h, negrow=np.full((1, 512), -1e30, f32),
            ones=ones, onesr=onesr, ident=ident,
        ))
    return in_maps, lens, pads, offs, total


def _build_nc(lens, pads, offs, total, reps=1):
    import concourse.mybir as mybir
    import concourse.tile as tile
    from concourse import bacc

    F32 = mybir.dt.float32
    F32R = mybir.dt.float32r
    AF = mybir.ActivationFunctionType
    OP = mybir.AluOpType

    nc = bacc.Bacc("TRN2", target_bir_lowering=False, debug=False,
                   num_devices=N_CORES)

    xT_d = nc.dram_tensor("xT", [HID, BS], F32R, kind="ExternalInput")
    wqT_d = nc.dram_tensor("wqT", [HID, EC], F32R, kind="ExternalInput")
    wkvT_d = nc.dram_tensor("wkvT", [HID, 2 * D], F32R, kind="ExternalInput")
    woT_d = nc.dram_tensor("woT", [EC, HID], F32R, kind="ExternalInput")
    kT_d = nc.dram_tensor("kT", [128, total], F32R, kind="ExternalInput")
    vC_d = nc.dram_tensor("vC", [total, 128], F32R, kind="ExternalInput")
    asin_d = nc.dram_tensor("a_sin", [128, BS], F32, kind="ExternalInput")
    acos_d = nc.dram_tensor("a_cos", [128, BS], F32, kind="ExternalInput")
    wqA_d = nc.dram_tensor("wqA", [128, 1], F32, kind="ExternalInput")
    wqB_d = nc.dram_tensor("wqB", [128, 1], F32, kind="ExternalInput")
    wkA_d = nc.dram_tensor("wkA", [128, 1], F32, kind="ExternalInput")
    wkB_d = nc.dram_tensor("wkB", [128, 1], F32, kind="ExternalInput")
    epsq_d = nc.dram_tensor("epsq", [1, 1], F32, kind="ExternalInput")
    epsk_d = nc.dram_tensor("epsk", [1, 1], F32, kind="ExternalInput")
    ones_d = nc.dram_tensor("ones", [128, 1], F32R, kind="ExternalInput")
    masks_d = nc.dram_tensor("masks", [1, 128 * B], F32R, kind="ExternalInput")
    negr_d = nc.dram_tensor("negrow", [1, 512], F32R, kind="ExternalInput")
    onesr_d = nc.dram_tensor("onesr", [1, 128], F32R, kind="ExternalInput")
    id_d = nc.dram_tensor("ident", [128, 128], F32, kind="ExternalInput")
    out_d = nc.dram_tensor("out", [BS, HID], F32, kind="ExternalOutput")

    with tile.TileContext(nc) as tc:
        with tc.tile_pool(name="const", bufs=1) as cpool, \
             tc.tile_pool(name="xpool", bufs=NDCH) as xpool, \
             tc.tile_pool(name="wpool", bufs=3) as wpool, \
             tc.tile_pool(name="persist", bufs=1) as ppool, \
             tc.tile_pool(name="work", bufs=2) as wk, \
             tc.tile_pool(name="probp", bufs=4) as probp, \
             tc.tile_pool(name="ktp", bufs=3) as ktp, \
             tc.tile_pool(name="vp", bufs=8) as vp, \
             tc.tile_pool(name="wop", bufs=8) as wop, \
             tc.tile_pool(name="acc", bufs=6, space="PSUM") as acc, \
             tc.tile_pool(name="rot", bufs=2, space="PSUM") as rot:

            ones = cpool.tile([128, 1], F32R, tag="ones")
            nc.sync.dma_start(out=ones[:, :], in_=ones_d[:, :])
            onesr = cpool.tile([1, 128], F32R, tag="onesr")
            nc.sync.dma_start(out=onesr[:, :], in_=onesr_d[:, :])
            ident = cpool.tile([128, 128], F32, tag="ident")
            nc.sync.dma_start(out=ident[:, :], in_=id_d[:, :])
            wqA = cpool.tile([128, 1], F32, tag="wqA")
            nc.sync.dma_start(out=wqA[:, :], in_=wqA_d[:, :])
            wqBt = cpool.tile([128, 1], F32, tag="wqB")
            nc.sync.dma_start(out=wqBt[:, :], in_=wqB_d[:, :])
            wkA = cpool.tile([128, 1], F32, tag="wkA")
            nc.sync.dma_start(out=wkA[:, :], in_=wkA_d[:, :])
            wkBt = cpool.tile([128, 1], F32, tag="wkB")
            nc.sync.dma_start(out=wkBt[:, :], in_=wkB_d[:, :])
            epsq = cpool.tile([1, 1], F32, tag="epsq")
            nc.sync.dma_start(out=epsq[:, :], in_=epsq_d[:, :])
            epsk = cpool.tile([1, 1], F32, tag="epsk")
            nc.sync.dma_start(out=epsk[:, :], in_=epsk_d[:, :])
            masks = cpool.tile([1, 128 * B], F32R, tag="masks")
            nc.sync.dma_start(out=masks[:, :], in_=masks_d[:, :])
            negrow = cpool.tile([1, 512], F32R, tag="negrow")
            nc.sync.dma_start(out=negrow[:, :], in_=negr_d[:, :])

            def rmsnorm_bcast(ps_in, sq_scale, bias_ap):
                """sumsq over partitions of ps_in -> 1/sqrt(scale*ss + bias)
                broadcast to [128, BS] in a rot-pool psum tile."""
                sq = wk.tile([128, BS], F32R, tag="sq")
                nc.scalar.activation(sq[:, :], ps_in[:, :], AF.Square)
                ps_ss = rot.tile([1, BS], F32, tag="rot")
                nc.tensor.matmul(ps_ss[:, :], ones[:, :], sq[:, :],
                                 start=True, stop=True)
                sqr = wk.tile([1, BS], F32, tag="sqr")
                nc.scalar.activation(sqr[:, :], ps_ss[:, :], AF.Sqrt,
                                     scale=sq_scale, bias=bias_ap[:, :])
                rstd = wk.tile([1, BS], F32R, tag="rstd")
                with nc.allow_low_precision(reason="f32r rounding"):
                    nc.vector.reciprocal(rstd[:, :], sqr[:, :])
                ps_bc = rot.tile([128, BS], F32, tag="rot")
                nc.tensor.matmul(ps_bc[:, :], onesr[:, :], rstd[:, :],
                                 start=True, stop=True)
                return ps_bc

            def rope_scale(ps_in, wA, wB, sin_t, cos_t, ps_bc, dst):
                """NeoX rope + per-token scale: dst[p, :] =
                (ps_in[p]*w[p]*cos[p] -/+ ps_in[partner]*w[partner]*sin[p])
                * bc[p, :]. dst is an f32r SBUF AP [128, BS]."""
                t_sb = wk.tile([128, BS], F32, tag="t_sb")
                nc.scalar.activation(t_sb[:, :], ps_in[:, :], AF.Copy)
                tswap = wk.tile([128, BS], F32, tag="tswap")
                nc.sync.dma_start(out=tswap[0:64, :], in_=t_sb[64:128, :])
                nc.sync.dma_start(out=tswap[64:128, :], in_=t_sb[0:64, :])
                m1 = wk.tile([128, BS], F32, tag="m1")
                nc.vector.scalar_tensor_tensor(
                    m1[:, :], t_sb[:, :], wA[:, :], cos_t[:, :],
                    op0=OP.mult, op1=OP.mult)
                m2 = wk.tile([128, BS], F32, tag="m2")
                nc.vector.scalar_tensor_tensor(
                    m2[:, :], tswap[:, :], wB[:, :], sin_t[:, :],
                    op0=OP.mult, op1=OP.mult)
                rt = wk.tile([128, BS], F32, tag="rt")
                nc.vector.tensor_sub(rt[0:64, :], m1[0:64, :], m2[0:64, :])
                nc.vector.tensor_add(rt[64:128, :], m1[64:128, :], m2[64:128, :])
                nc.vector.tensor_mul(dst, rt[:, :], ps_bc[:, :])

            def body(_it):
                # ---- load x chunks (resident, shared by all projections) ----
                xts = []
                for dch in range(NDCH):
                    xt = xpool.tile([128, BS], F32R, tag="xt")
                    nc.sync.dma_start(out=xt[:, :],
                                      in_=xT_d[dch * 128:(dch + 1) * 128, :])
                    xts.append(xt)

                # ---- rope tables ----
                asin = wk.tile([128, BS], F32, tag="asin")
                nc.sync.dma_start(out=asin[:, :], in_=asin_d[:, :])
                acos = wk.tile([128, BS], F32, tag="acos")
                nc.sync.dma_start(out=acos[:, :], in_=acos_d[:, :])
                sin_t = ppool.tile([128, BS], F32, tag="sin")
                nc.scalar.activation(sin_t[:, :], asin[:, :], AF.Sin)
                cos_t = ppool.tile([128, BS], F32, tag="cos")
                nc.scalar.activation(cos_t[:, :], acos[:, :], AF.Sin)

                # ---- phase A1: K,V projections ----
                ps_k = acc.tile([128, BS], F32, tag="acc")
                ps_v = acc.tile([128, BS], F32, tag="acc")
                for dch in range(NDCH):
                    wkv = wpool.tile([128, 2 * D], F32R, tag="wkv")
                    nc.sync.dma_start(out=wkv[:, :],
                                      in_=wkvT_d[dch * 128:(dch + 1) * 128, :])
                    nc.tensor.matmul(ps_k[:, :], wkv[:, 0:D], xts[dch][:, :],
                                     start=(dch == 0), stop=(dch == NDCH - 1))
                    nc.tensor.matmul(ps_v[:, :], wkv[:, D:2 * D], xts[dch][:, :],
                                     start=(dch == 0), stop=(dch == NDCH - 1))

                # ---- k: rmsnorm + rope -> k_sb [D, BS] f32r ----
                ps_bc_k = rmsnorm_bcast(ps_k, 1.0 / D, epsk)
                k_sb = ppool.tile([128, BS], F32R, tag="k_sb")
                rope_scale(ps_k, wkA, wkBt, sin_t, cos_t, ps_bc_k, k_sb[:, :])

                # ---- v: transpose to [token, D] per batch -> vt_sb ----
                v_tmp = ppool.tile([128, BS], F32, tag="v_tmp")
                nc.scalar.activation(v_tmp[:, :], ps_v[:, :], AF.Copy)
                vt_sb = ppool.tile([128, BS], F32R, tag="vt_sb")
                for b in range(B):
                    ps_vt = rot.tile([128, 128], F32, tag="rot")
                    nc.tensor.transpose(ps_vt[:, :],
                                        v_tmp[:, b * S:(b + 1) * S],
                                        ident[:, :])
                    nc.vector.tensor_copy(vt_sb[:, b * S:(b + 1) * S],
                                          ps_vt[:, :])

                # ---- phase A2: Q projections + norm + rope ----
                q_sb = ppool.tile([128, HQC * BS], F32R, tag="q_sb")
                ps_qs = [acc.tile([128, BS], F32, tag="acc", name=f"ps_q{h}") for h in range(HQC)]
                for dch in range(NDCH):
                    wq = wpool.tile([128, EC], F32R, tag="wq")
                    nc.sync.dma_start(out=wq[:, :],
                                      in_=wqT_d[dch * 128:(dch + 1) * 128, :])
                    for h in range(HQC):
                        nc.tensor.matmul(ps_qs[h][:, :],
                                         wq[:, h * D:(h + 1) * D],
                                         xts[dch][:, :],
                                         start=(dch == 0), stop=(dch == NDCH - 1))
                for h in range(HQC):
                    # fold the attention scale into the rmsnorm scale:
                    # rstd*SCALE = 1/sqrt(ss + D*eps)
                    ps_bc_q = rmsnorm_bcast(ps_qs[h], 1.0, epsq)
                    rope_scale(ps_qs[h], wqA, wqBt, sin_t, cos_t, ps_bc_q,
                               q_sb[:, h * BS:(h + 1) * BS])

                # q viewed as [128, h, b, s] for per-batch 3D moving operands
                q4 = q_sb.rearrange("p (h b s) -> p h b s", h=HQC, b=B)

                # ---- attention per batch ----
                o_sb = ppool.tile([128, B * 512], F32R, tag="o_sb")
                for b in range(B):
                    ncache = pads[b] // 128
                    nch = ncache + 1
                    tail = lens[b] - (ncache - 1) * 128 if ncache > 0 else 0
                    ps_o = acc.tile([128, 512], F32, tag="acc")
                    ps_sum = acc.tile([1, 512], F32, tag="acc")
                    pending = []

                    def flush(ci_, prob_):
                        nc.tensor.matmul(ps_o[:, :], vtile[ci_], prob_[:, :],
                                         start=(ci_ == 0), stop=(ci_ == nch - 1))
                        nc.tensor.matmul(ps_sum[:, :], ones[:, :], prob_[:, :],
                                         start=(ci_ == 0), stop=(ci_ == nch - 1))

                    vtile = {}
                    for ci in range(nch):
                        is_new = ci == ncache
                        # stationary K chunk
                        if is_new:
                            k_chunk = k_sb[:, b * S:(b + 1) * S]
                            vtile[ci] = vt_sb[:, b * S:(b + 1) * S]
                        else:
                            kt = ktp.tile([128, 128], F32R, tag="kt")
                            nc.sync.dma_start(
                                out=kt[:, :],
                                in_=kT_d[:, offs[b] + ci * 128:
                                         offs[b] + (ci + 1) * 128])
                            k_chunk = kt[:, :]
                            vt = vp.tile([128, 128], F32R, tag="v")
                            nc.sync.dma_start(
                                out=vt[:, :],
                                in_=vC_d[offs[b] + ci * 128:
                                         offs[b] + (ci + 1) * 128, :])
                            vtile[ci] = vt[:, :]
                        ps_s = rot.tile([128, 512], F32, tag="rot")
                        masked = ((not is_new) and ci == ncache - 1
                                  and tail < 128)
                        nc.tensor.matmul(ps_s[:, :], k_chunk,
                                         q4[:, :, b, :], start=True,
                                         stop=not masked)
                        if masked:
                            nc.tensor.matmul(
                                ps_s[:, :], masks[:, b * 128:(b + 1) * 128],
                                negrow[:, :], start=False, stop=True)
                        prob = probp.tile([128, 512], F32R, tag="prob")
                        nc.scalar.activation(prob[:, :], ps_s[:, :], AF.Exp)
                        pending.append((ci, prob))
                        if len(pending) > 1:
                            flush(*pending.pop(0))
                    while pending:
                        flush(*pending.pop(0))

                    # normalize: o * (1/sum) broadcast
                    rec = wk.tile([1, 512], F32R, tag="rec")
                    with nc.allow_low_precision(reason="f32r rounding"):
                        nc.vector.reciprocal(rec[:, :], ps_sum[:, :])
                    ps_bc2 = rot.tile([128, 512], F32, tag="rot")
                    nc.tensor.matmul(ps_bc2[:, :], onesr[:, :], rec[:, :],
                                     start=True, stop=True)
                    bc_sb = wk.tile([128, 512], F32, tag="bc_sb")
                    nc.scalar.activation(bc_sb[:, :], ps_bc2[:, :], AF.Copy)
                    nc.vector.tensor_mul(o_sb[:, b * 512:(b + 1) * 512],
                                         ps_o[:, :], bc_sb[:, :])

                # ---- output projection: partial = o @ WoT_c ----
                for hc in range(HID // 512):
                    wos = []
                    for h in range(HQC):
                        wo = wop.tile([128, 512], F32R, tag="wo")
                        nc.sync.dma_start(
                            out=wo[:, :],
                            in_=woT_d[h * D:(h + 1) * D,
                                      hc * 512:(hc + 1) * 512])
                        wos.append(wo)
                    for b in range(B):
                        ps_out = acc.tile([128, 512], F32, tag="acc")
                        for h in range(HQC):
                            nc.tensor.matmul(
                                ps_out[:, :],
                                o_sb[:, b * 512 + h * D:b * 512 + (h + 1) * D],
                                wos[h][:, :],
                                start=(h == 0), stop=(h == HQC - 1))
                        od = wk.tile([128, 512], F32, tag="od")
                        nc.scalar.activation(od[:, :], ps_out[:, :], AF.Copy)
                        nc.sync.dma_start(
                            out=out_d[b * S:(b + 1) * S,
                                      hc * 512:(hc + 1) * 512],
                            in_=od[:, :])

            if reps == 1:
                body(0)
            else:
                with tc.For_i(0, reps, 1) as it:
                    body(it)

    nc.compile()
    return nc


def _get_nc(lens, pads, offs, total, reps=1):
    key = (tuple(lens), total, reps)
    if key not in _CACHE:
        _CACHE[key] = _build_nc(lens, pads, offs, total, reps)
    return _CACHE[key]


def kernel(x, Wq, Wk, Wv, Wo, q_norm_w, k_norm_w, k_cache, v_cache,
           block_table, cache_seqlens):
    from concourse.bass_utils import run_bass_kernel_spmd

    in_maps, lens, pads, offs, total = _prep_host(
        x, Wq, Wk, Wv, Wo, q_norm_w, k_norm_w, k_cache, v_cache,
        block_table, cache_seqlens)
    nc = _get_nc(lens, pads, offs, total, reps=1)
    res = run_bass_kernel_spmd(nc, in_maps, core_ids=list(range(N_CORES)))
    partials = np.stack([r["out"] for r in res.results], 0)
    out = np.sum(partials, axis=0, dtype=np.float64).astype(np.float32)
    return out.reshape(B, S, HID)


# revision 13
# speedup vs baseline: 1.0410x; 1.0410x over previous
"""DFlashAttention (paged KV cache decode-attention block) on 8 Trainium2
NeuronCores.

Sharding: tensor-parallel over heads. Each core owns HQ/8 = 4 query heads and
HK/8 = 1 KV head (GQA group). Wq/Wk/Wv row-sharded, Wo column-sharded; each
core produces a partial output [B*S, HID] which is reduced on the host.

Device kernel layout choices:
  - All big matmuls in float32r (full PE rate, ~1e-4 relative rounding).
  - Projections produce q/k/v in [D, token] layout (head dim on partitions).
  - Scores are computed transposed: [l_chunk(128), (head, s)=512] with the
    KV-cache chunk as the stationary operand, so softmax-sum reduces over
    partitions via a ones-matmul and PV consumes probs directly - no
    transposes anywhere in the attention inner loop.
  - Per-batch cache lengths are baked into the instruction stream at build
    time (kernel() sees cache_seqlens on the host); the final partial cache
    chunk is masked by accumulating a rank-1 (-1e30) outer product into the
    scores so exp underflows to exactly zero.
  - RMSNorm per-token scales are folded in as column broadcasts via rank-1
    ones-matmuls; RoPE uses two half-swap DMAs per head plus
    scalar_tensor_tensor ops (keeps every DVE op lane-aligned).
  - DMA traffic is batched into few multi-chunk descriptors on the sync
    engine (descriptor generation is ~0.8us each); per-chunk V tiles, the
    rope half-swaps and the output stores issue from GpSimd, which is
    otherwise idle.
"""

import sys

sys.path.insert(0, "/opt/trn_rl_repo")

import numpy as np

B, S, HID = 4, 128, 4096
D, HQ, HK = 128, 32, 8
PAGES, PSIZE, NPP = 64, 256, 16
THETA = 10000.0
EPS = 1e-6
N_CORES = 8
HQC = HQ // N_CORES  # 4 query heads per core
EC = HQC * D         # 512 output-proj contraction per core
BS = B * S           # 512 tokens
NDCH = HID // 128    # 32 contraction chunks for projections

_CACHE = {}


def _prep_host(x, Wq, Wk, Wv, Wo, q_norm_w, k_norm_w, k_cache, v_cache,
               block_table, cache_seqlens):
    f32 = np.float32
    xT = np.ascontiguousarray(np.asarray(x, f32).reshape(BS, HID).T)

    lens = [int(v) for v in np.asarray(cache_seqlens)]
    pads = [(l + 127) // 128 * 128 for l in lens]
    offs = [0] * B
    for b in range(1, B):
        offs[b] = offs[b - 1] + pads[b - 1]
    total = max(sum(pads), 128)

    bt = np.asarray(block_table)
    kg = np.asarray(k_cache, f32)[bt].reshape(B, NPP * PSIZE, HK, D)
    vg = np.asarray(v_cache, f32)[bt].reshape(B, NPP * PSIZE, HK, D)

    # RoPE angles, range-reduced to [-pi, pi) on the host (index arithmetic
    # only; sin/cos evaluated on device). Mimic the reference's fp32 freqs.
    pos = np.asarray(cache_seqlens, np.float64)[:, None] + np.arange(S)[None, :]
    inv = 1.0 / (THETA ** (np.arange(0, D, 2, dtype=np.float64) / D))
    freqs32 = (pos.astype(f32)[:, :, None] * inv.astype(f32)[None, None, :]).astype(f32)
    fr = np.float64(freqs32)
    two_pi = 2 * np.pi

    def red(a):
        m = np.mod(a, two_pi)
        m = np.where(m >= np.pi, m - two_pi, m)
        return m.astype(f32)

    a_sin = red(fr).reshape(BS, 64).T                      # [64, BS]
    a_cos = red(fr + np.pi / 2).reshape(BS, 64).T
    # duplicate across both partition halves -> [128, BS]
    a_sin2 = np.ascontiguousarray(np.concatenate([a_sin, a_sin], 0))
    a_cos2 = np.ascontiguousarray(np.concatenate([a_cos, a_cos], 0))

    wq = np.asarray(q_norm_w, f32).reshape(D, 1)
    wk = np.asarray(k_norm_w, f32).reshape(D, 1)
    wqB = np.roll(wq, 64, axis=0).copy()   # partner-half weight
    wkB = np.roll(wk, 64, axis=0).copy()

    masks_h = np.zeros((1, 128 * B), f32)
    for b in range(B):
        t = lens[b] - (pads[b] // 128 - 1) * 128 if pads[b] > 0 else 128
        masks_h[0, b * 128 + t:(b + 1) * 128] = 1.0

    Wq_ = np.asarray(Wq, f32)
    Wk_ = np.asarray(Wk, f32)
    Wv_ = np.asarray(Wv, f32)
    Wo_ = np.asarray(Wo, f32)

    in_maps = []
    for c in range(N_CORES):
        wqT = np.ascontiguousarray(Wq_[c * EC:(c + 1) * EC, :].T)     # [HID, 512]
        wkvT = np.ascontiguousarray(
            np.concatenate([Wk_[c * D:(c + 1) * D, :],
                            Wv_[c * D:(c + 1) * D, :]], 0).T)         # [HID, 256]
        woT = np.ascontiguousarray(Wo_[:, c * EC:(c + 1) * EC].T)     # [512, HID]
        kT = np.zeros((128, total), f32)
        vC = np.zeros((total, 128), f32)
        for b in range(B):
            nb, ob = lens[b], offs[b]
            if nb > 0:
                kT[:, ob:ob + nb] = kg[b, :nb, c, :].T
                vC[ob:ob + nb, :] = vg[b, :nb, c, :]
        in_maps.append(dict(
            xT=xT, wqT=wqT, wkvT=wkvT, woT=woT,
            kT=np.ascontiguousarray(kT), vC=vC,
            a_sin=a_sin2, a_cos=a_cos2,
            wqA=wq, wqB=wqB, wkA=wk, wkB=wkB,
            epsq=np.full((1, 1), D * EPS, f32),
            epsk=np.full((1, 1), EPS, f32),
            masks=masks_h, negrow=np.full((1, 512), -1e30, f32),
            ones=np.ones((128, 1), f32), onesr=np.ones((1, 128), f32),
            ident=np.eye(128, dtype=f32),
        ))
    return in_maps, lens, pads, offs, total


def _build_nc(lens, pads, offs, total, reps=1):
    import concourse.mybir as mybir
    import concourse.tile as tile
    from concourse import bacc

    F32 = mybir.dt.float32
    F32R = mybir.dt.float32r
    AF = mybir.ActivationFunctionType
    OP = mybir.AluOpType

    nc = bacc.Bacc("TRN2", target_bir_lowering=False, debug=False,
                   num_devices=N_CORES)

    xT_d = nc.dram_tensor("xT", [HID, BS], F32R, kind="ExternalInput")
    wqT_d = nc.dram_tensor("wqT", [HID, EC], F32R, kind="ExternalInput")
    wkvT_d = nc.dram_tensor("wkvT", [HID, 2 * D], F32R, kind="ExternalInput")
    woT_d = nc.dram_tensor("woT", [EC, HID], F32R, kind="ExternalInput")
    kT_d = nc.dram_tensor("kT", [128, total], F32R, kind="ExternalInput")
    vC_d = nc.dram_tensor("vC", [total, 128], F32R, kind="ExternalInput")
    asin_d = nc.dram_tensor("a_sin", [128, BS], F32, kind="ExternalInput")
    acos_d = nc.dram_tensor("a_cos", [128, BS], F32, kind="ExternalInput")
    wqA_d = nc.dram_tensor("wqA", [128, 1], F32, kind="ExternalInput")
    wqB_d = nc.dram_tensor("wqB", [128, 1], F32, kind="ExternalInput")
    wkA_d = nc.dram_tensor("wkA", [128, 1], F32, kind="ExternalInput")
    wkB_d = nc.dram_tensor("wkB", [128, 1], F32, kind="ExternalInput")
    epsq_d = nc.dram_tensor("epsq", [1, 1], F32, kind="ExternalInput")
    epsk_d = nc.dram_tensor("epsk", [1, 1], F32, kind="ExternalInput")
    masks_d = nc.dram_tensor("masks", [1, 128 * B], F32R, kind="ExternalInput")
    negr_d = nc.dram_tensor("negrow", [1, 512], F32R, kind="ExternalInput")
    ones_d = nc.dram_tensor("ones", [128, 1], F32R, kind="ExternalInput")
    onesr_d = nc.dram_tensor("onesr", [1, 128], F32R, kind="ExternalInput")
    id_d = nc.dram_tensor("ident", [128, 128], F32, kind="ExternalInput")
    out_d = nc.dram_tensor("out", [BS, HID], F32, kind="ExternalOutput")

    # DRAM big-views for batched loads: row-chunk c, partition p, col e.
    xT_v = xT_d.rearrange("(c p) e -> p c e", p=128)       # [128, 32, 512]
    wqT_v = wqT_d.rearrange("(c p) e -> p c e", p=128)     # [128, 32, 512]
    wkvT_v = wkvT_d.rearrange("(c p) e -> p c e", p=128)   # [128, 32, 256]
    woT_v = woT_d.rearrange("(c p) e -> p c e", p=128)     # [128, 4, 4096]

    with tile.TileContext(nc) as tc:
        with tc.tile_pool(name="const", bufs=1) as cpool, \
             tc.tile_pool(name="xpool", bufs=3) as xpool, \
             tc.tile_pool(name="wpool", bufs=3) as wpool, \
             tc.tile_pool(name="persist", bufs=1) as ppool, \
             tc.tile_pool(name="work", bufs=2) as wk, \
             tc.tile_pool(name="probp", bufs=4) as probp, \
             tc.tile_pool(name="ktp", bufs=2) as ktp, \
             tc.tile_pool(name="vp", bufs=8) as vp, \
             tc.tile_pool(name="wop", bufs=8) as wop, \
             tc.tile_pool(name="acc", bufs=6, space="PSUM") as acc, \
             tc.tile_pool(name="rot", bufs=2, space="PSUM") as rot:

            ones = cpool.tile([128, 1], F32R, tag="ones")
            nc.gpsimd.dma_start(out=ones[:, :], in_=ones_d[:, :])
            onesr = cpool.tile([1, 128], F32R, tag="onesr")
            nc.gpsimd.dma_start(out=onesr[:, :], in_=onesr_d[:, :])
            ident = cpool.tile([128, 128], F32, tag="ident")
            nc.gpsimd.dma_start(out=ident[:, :], in_=id_d[:, :])
            wqA = cpool.tile([128, 1], F32, tag="wqA")
            nc.gpsimd.dma_start(out=wqA[:, :], in_=wqA_d[:, :])
            wqBt = cpool.tile([128, 1], F32, tag="wqB")
            nc.gpsimd.dma_start(out=wqBt[:, :], in_=wqB_d[:, :])
            wkA = cpool.tile([128, 1], F32, tag="wkA")
            nc.gpsimd.dma_start(out=wkA[:, :], in_=wkA_d[:, :])
            wkBt = cpool.tile([128, 1], F32, tag="wkB")
            nc.gpsimd.dma_start(out=wkBt[:, :], in_=wkB_d[:, :])
            epsq = cpool.tile([1, 1], F32, tag="epsq")
            nc.gpsimd.dma_start(out=epsq[:, :], in_=epsq_d[:, :])
            epsk = cpool.tile([1, 1], F32, tag="epsk")
            nc.gpsimd.dma_start(out=epsk[:, :], in_=epsk_d[:, :])
            masks = cpool.tile([1, 128 * B], F32R, tag="masks")
            nc.gpsimd.dma_start(out=masks[:, :], in_=masks_d[:, :])
            negrow = cpool.tile([1, 512], F32R, tag="negrow")
            nc.gpsimd.dma_start(out=negrow[:, :], in_=negr_d[:, :])

            def rmsnorm_bcast(ps_in, sq_scale, bias_ap):
                """sumsq over partitions of ps_in -> 1/sqrt(scale*ss + bias)
                broadcast to [128, BS] in a rot-pool psum tile."""
                sq = wk.tile([128, BS], F32R, tag="sq")
                nc.scalar.activation(sq[:, :], ps_in[:, :], AF.Square)
                ps_ss = rot.tile([1, BS], F32, tag="rot")
                nc.tensor.matmul(ps_ss[:, :], ones[:, :], sq[:, :],
                                 start=True, stop=True)
                sqr = wk.tile([1, BS], F32, tag="sqr")
                nc.scalar.activation(sqr[:, :], ps_ss[:, :], AF.Sqrt,
                                     scale=sq_scale, bias=bias_ap[:, :])
                rstd = wk.tile([1, BS], F32R, tag="rstd")
                with nc.allow_low_precision(reason="f32r rounding"):
                    nc.vector.reciprocal(rstd[:, :], sqr[:, :])
                ps_bc = rot.tile([128, BS], F32, tag="rot")
                nc.tensor.matmul(ps_bc[:, :], onesr[:, :], rstd[:, :],
                                 start=True, stop=True)
                return ps_bc

            def rope_scale(ps_in, wA, wB, sin_t, cos_t, ps_bc, dst):
                """NeoX rope + per-token scale: dst[p, :] =
                (ps_in[p]*w[p]*cos[p] -/+ ps_in[partner]*w[partner]*sin[p])
                * bc[p, :]. dst is an f32r SBUF AP [128, BS]."""
                t_sb = wk.tile([128, BS], F32, tag="t_sb")
                nc.scalar.activation(t_sb[:, :], ps_in[:, :], AF.Copy)
                tswap = wk.tile([128, BS], F32, tag="tswap")
                nc.gpsimd.dma_start(out=tswap[0:64, :], in_=t_sb[64:128, :])
                nc.gpsimd.dma_start(out=tswap[64:128, :], in_=t_sb[0:64, :])
                m1 = wk.tile([128, BS], F32, tag="m1")
                nc.vector.scalar_tensor_tensor(
                    m1[:, :], t_sb[:, :], wA[:, :], cos_t[:, :],
                    op0=OP.mult, op1=OP.mult)
                m2 = wk.tile([128, BS], F32, tag="m2")
                nc.vector.scalar_tensor_tensor(
                    m2[:, :], tswap[:, :], wB[:, :], sin_t[:, :],
                    op0=OP.mult, op1=OP.mult)
                rt = wk.tile([128, BS], F32, tag="rt")
                nc.vector.tensor_sub(rt[0:64, :], m1[0:64, :], m2[0:64, :])
                nc.vector.tensor_add(rt[64:128, :], m1[64:128, :], m2[64:128, :])
                nc.vector.tensor_mul(dst, rt[:, :], ps_bc[:, :])

            def body(_it):
                # ---- rope tables ----
                asin = wk.tile([128, BS], F32, tag="asin")
                nc.gpsimd.dma_start(out=asin[:, :], in_=asin_d[:, :])
                acos = wk.tile([128, BS], F32, tag="acos")
                nc.gpsimd.dma_start(out=acos[:, :], in_=acos_d[:, :])
                sin_t = ppool.tile([128, BS], F32, tag="sin")
                nc.scalar.activation(sin_t[:, :], asin[:, :], AF.Sin)
                cos_t = ppool.tile([128, BS], F32, tag="cos")
                nc.scalar.activation(cos_t[:, :], acos[:, :], AF.Sin)

                # ---- phase A: Q,K,V projections in one streamed pass ----
                ps_k = acc.tile([128, BS], F32, tag="acc")
                ps_v = acc.tile([128, BS], F32, tag="acc")
                ps_qs = [acc.tile([128, BS], F32, tag="acc", name=f"ps_q{h}")
                         for h in range(HQC)]
                GRP = 4  # d-chunks per DMA group
                for g in range(NDCH // GRP):
                    xtile = xpool.tile([128, GRP * BS], F32R, tag="xt")
                    nc.sync.dma_start(out=xtile[:, :],
                                      in_=xT_v[:, g * GRP:(g + 1) * GRP, :])
                    wq = wpool.tile([128, GRP * EC], F32R, tag="wq")
                    nc.scalar.dma_start(out=wq[:, :],
                                      in_=wqT_v[:, g * GRP:(g + 1) * GRP, :])
                    wkv = wpool.tile([128, GRP * 256], F32R, tag="wkv")
                    nc.gpsimd.dma_start(out=wkv[:, :],
                                      in_=wkvT_v[:, g * GRP:(g + 1) * GRP, :])
                    for j in range(GRP):
                        dch = g * GRP + j
                        st = dch == 0
                        sp = dch == NDCH - 1
                        xa = xtile[:, j * BS:(j + 1) * BS]
                        nc.tensor.matmul(ps_k[:, :], wkv[:, j * 256:j * 256 + D],
                                         xa, start=st, stop=sp)
                        nc.tensor.matmul(ps_v[:, :],
                                         wkv[:, j * 256 + D:(j + 1) * 256],
                                         xa, start=st, stop=sp)
                        for h in range(HQC):
                            nc.tensor.matmul(
                                ps_qs[h][:, :],
                                wq[:, j * EC + h * D:j * EC + (h + 1) * D],
                                xa, start=st, stop=sp)

                # ---- k: rmsnorm + rope -> k_sb [D, BS] f32r ----
                ps_bc_k = rmsnorm_bcast(ps_k, 1.0 / D, epsk)
                k_sb = ppool.tile([128, BS], F32R, tag="k_sb")
                rope_scale(ps_k, wkA, wkBt, sin_t, cos_t, ps_bc_k, k_sb[:, :])

                # ---- v: transpose to [token, D] per batch -> vt_sb ----
                v_tmp = ppool.tile([128, BS], F32, tag="v_tmp")
                nc.scalar.activation(v_tmp[:, :], ps_v[:, :], AF.Copy)
                vt_sb = ppool.tile([128, BS], F32R, tag="vt_sb")
                for b in range(B):
                    ps_vt = rot.tile([128, 128], F32, tag="rot")
                    nc.tensor.transpose(ps_vt[:, :],
                                        v_tmp[:, b * S:(b + 1) * S],
                                        ident[:, :])
                    nc.vector.tensor_copy(vt_sb[:, b * S:(b + 1) * S],
                                          ps_vt[:, :])

                # ---- q: norm + rope per head ----
                q_sb = ppool.tile([128, HQC * BS], F32R, tag="q_sb")
                for h in range(HQC):
                    # fold the attention scale into the rmsnorm scale:
                    # rstd*SCALE = 1/sqrt(ss + D*eps)
                    ps_bc_q = rmsnorm_bcast(ps_qs[h], 1.0, epsq)
                    rope_scale(ps_qs[h], wqA, wqBt, sin_t, cos_t, ps_bc_q,
                               q_sb[:, h * BS:(h + 1) * BS])

                # q viewed as [128, h, b, s] for per-batch 3D moving operands
                q4 = q_sb.rearrange("p (h b s) -> p h b s", h=HQC, b=B)

                # ---- attention per batch ----
                o_sb = ppool.tile([128, B * 512], F32R, tag="o_sb")
                for b in range(B):
                    ncache = pads[b] // 128
                    nch = ncache + 1
                    tail = lens[b] - (ncache - 1) * 128 if ncache > 0 else 0
                    ps_o = acc.tile([128, 512], F32, tag="acc")
                    ps_sum = acc.tile([1, 512], F32, tag="acc")
                    pending = []

                    def flush(ci_, prob_):
                        nc.tensor.matmul(ps_o[:, :], vtile[ci_], prob_[:, :],
                                         start=(ci_ == 0), stop=(ci_ == nch - 1))
                        nc.tensor.matmul(ps_sum[:, :], ones[:, :], prob_[:, :],
                                         start=(ci_ == 0), stop=(ci_ == nch - 1))

                    vtile = {}
                    # batched K loads for this batch: groups of 8 chunks
                    kts = []
                    for g in range((ncache + 7) // 8):
                        c0 = g * 8
                        c1 = min(ncache, c0 + 8)
                        kt = ktp.tile([128, 1024], F32R, tag="kt")
                        nc.sync.dma_start(
                            out=kt[:, :(c1 - c0) * 128],
                            in_=kT_d[:, offs[b] + c0 * 128:offs[b] + c1 * 128])
                        kts.append(kt)
                    for ci in range(nch):
                        is_new = ci == ncache
                        if is_new:
                            k_chunk = k_sb[:, b * S:(b + 1) * S]
                            vtile[ci] = vt_sb[:, b * S:(b + 1) * S]
                        else:
                            k_chunk = kts[ci // 8][:, (ci % 8) * 128:
                                                   (ci % 8 + 1) * 128]
                            vt = vp.tile([128, 128], F32R, tag="v")
                            nc.gpsimd.dma_start(
                                out=vt[:, :],
                                in_=vC_d[offs[b] + ci * 128:
                                         offs[b] + (ci + 1) * 128, :])
                            vtile[ci] = vt[:, :]
                        ps_s = rot.tile([128, 512], F32, tag="rot")
                        masked = ((not is_new) and ci == ncache - 1
                                  and tail < 128)
                        nc.tensor.matmul(ps_s[:, :], k_chunk,
                                         q4[:, :, b, :], start=True,
                                         stop=not masked)
                        if masked:
                            nc.tensor.matmul(
                                ps_s[:, :], masks[:, b * 128:(b + 1) * 128],
                                negrow[:, :], start=False, stop=True)
                        prob = probp.tile([128, 512], F32R, tag="prob")
                        nc.scalar.activation(prob[:, :], ps_s[:, :], AF.Exp)
                        pending.append((ci, prob))
                        if len(pending) > 1:
                            flush(*pending.pop(0))
                    while pending:
                        flush(*pending.pop(0))

                    # normalize: o * (1/sum) broadcast
                    rec = wk.tile([1, 512], F32R, tag="rec")
                    with nc.allow_low_precision(reason="f32r rounding"):
                        nc.vector.reciprocal(rec[:, :], ps_sum[:, :])
                    ps_bc2 = rot.tile([128, 512], F32, tag="rot")
                    nc.tensor.matmul(ps_bc2[:, :], onesr[:, :], rec[:, :],
                                     start=True, stop=True)
                    bc_sb = wk.tile([128, 512], F32, tag="bc_sb")
                    nc.scalar.activation(bc_sb[:, :], ps_bc2[:, :], AF.Copy)
                    nc.vector.tensor_mul(o_sb[:, b * 512:(b + 1) * 512],
                                         ps_o[:, :], bc_sb[:, :])

                # ---- output projection: partial = o @ WoT_c ----
                for quarter in range(4):
                    wos = []
                    for h in range(HQC):
                        wo = wop.tile([128, 1024], F32R, tag="wo",
                                      name=f"wo{quarter}_{h}")
                        nc.sync.dma_start(
                            out=wo[:, :],
                            in_=woT_v[:, h, quarter * 1024:(quarter + 1) * 1024])
                        wos.append(wo)
                    for hc in range(2):
                        for b in range(B):
                            ps_out = acc.tile([128, 512], F32, tag="acc")
                            for h in range(HQC):
                                nc.tensor.matmul(
                                    ps_out[:, :],
                                    o_sb[:, b * 512 + h * D:
                                         b * 512 + (h + 1) * D],
                                    wos[h][:, hc * 512:(hc + 1) * 512],
                                    start=(h == 0), stop=(h == HQC - 1))
                            od = wk.tile([128, 512], F32, tag="od")
                            if (hc * B + b) % 2 == 0:
                                nc.vector.tensor_copy(od[:, :], ps_out[:, :])
                            else:
                                nc.scalar.activation(od[:, :], ps_out[:, :],
                                                     AF.Copy)
                            nc.gpsimd.dma_start(
                                out=out_d[b * S:(b + 1) * S,
                                          quarter * 1024 + hc * 512:
                                          quarter * 1024 + (hc + 1) * 512],
                                in_=od[:, :])

            if reps == 1:
                body(0)
            else:
                with tc.For_i(0, reps, 1) as it:
                    body(it)

    nc.compile()
    return nc


def _get_nc(lens, pads, offs, total, reps=1):
    key = (tuple(lens), total, reps)
    if key not in _CACHE:
        _CACHE[key] = _build_nc(lens, pads, offs, total, reps)
    return _CACHE[key]


def kernel(x, Wq, Wk, Wv, Wo, q_norm_w, k_norm_w, k_cache, v_cache,
           block_table, cache_seqlens):
    from concourse.bass_utils import run_bass_kernel_spmd

    in_maps, lens, pads, offs, total = _prep_host(
        x, Wq, Wk, Wv, Wo, q_norm_w, k_norm_w, k_cache, v_cache,
        block_table, cache_seqlens)
    nc = _get_nc(lens, pads, offs, total, reps=1)
    res = run_bass_kernel_spmd(nc, in_maps, core_ids=list(range(N_CORES)))
    partials = np.stack([r["out"] for r in res.results], 0)
    out = np.sum(partials, axis=0, dtype=np.float64).astype(np.float32)
    return out.reshape(B, S, HID)


# revision 14
# speedup vs baseline: 1.1030x; 1.0596x over previous
"""DFlashAttention (paged KV cache decode-attention block) on 8 Trainium2
NeuronCores.

Sharding: tensor-parallel over heads. Each core owns HQ/8 = 4 query heads and
HK/8 = 1 KV head (GQA group). Wq/Wk/Wv row-sharded, Wo column-sharded; each
core produces a partial output [B*S, HID] which is reduced on the host.

Device kernel layout choices:
  - All big matmuls in float32r (full PE rate, ~1e-4 relative rounding).
  - Projections produce q/k/v in [D, token] layout (head dim on partitions).
  - Scores are computed transposed: [l_chunk(128), (head, s)=512] with the
    KV-cache chunk as the stationary operand, so softmax-sum reduces over
    partitions via a ones-matmul and PV consumes probs directly - no
    transposes anywhere in the attention inner loop.
  - Per-batch cache lengths are baked into the instruction stream at build
    time (kernel() sees cache_seqlens on the host); the final partial cache
    chunk is masked by accumulating a rank-1 (-1e30) outer product into the
    scores so exp underflows to exactly zero.
  - RMSNorm per-token scales are folded in as column broadcasts via rank-1
    ones-matmuls; RoPE uses two half-swap DMAs per head plus
    scalar_tensor_tensor ops (keeps every DVE op lane-aligned).
  - DMA traffic is batched into few multi-chunk descriptors on the sync
    engine (descriptor generation is ~0.8us each); per-chunk V tiles, the
    rope half-swaps and the output stores issue from GpSimd, which is
    otherwise idle.
"""

import sys

sys.path.insert(0, "/opt/trn_rl_repo")

import numpy as np

B, S, HID = 4, 128, 4096
D, HQ, HK = 128, 32, 8
PAGES, PSIZE, NPP = 64, 256, 16
THETA = 10000.0
EPS = 1e-6
N_CORES = 8
HQC = HQ // N_CORES  # 4 query heads per core
EC = HQC * D         # 512 output-proj contraction per core
BS = B * S           # 512 tokens
NDCH = HID // 128    # 32 contraction chunks for projections

_CACHE = {}


def _prep_host(x, Wq, Wk, Wv, Wo, q_norm_w, k_norm_w, k_cache, v_cache,
               block_table, cache_seqlens):
    f32 = np.float32
    xT = np.ascontiguousarray(np.asarray(x, f32).reshape(BS, HID).T)

    lens = [int(v) for v in np.asarray(cache_seqlens)]
    pads = [(l + 127) // 128 * 128 for l in lens]
    offs = [0] * B
    for b in range(1, B):
        offs[b] = offs[b - 1] + pads[b - 1]
    total = max(sum(pads), 128)

    bt = np.asarray(block_table)
    kg = np.asarray(k_cache, f32)[bt].reshape(B, NPP * PSIZE, HK, D)
    vg = np.asarray(v_cache, f32)[bt].reshape(B, NPP * PSIZE, HK, D)

    # RoPE angles, range-reduced to [-pi, pi) on the host (index arithmetic
    # only; sin/cos evaluated on device). Mimic the reference's fp32 freqs.
    pos = np.asarray(cache_seqlens, np.float64)[:, None] + np.arange(S)[None, :]
    inv = 1.0 / (THETA ** (np.arange(0, D, 2, dtype=np.float64) / D))
    freqs32 = (pos.astype(f32)[:, :, None] * inv.astype(f32)[None, None, :]).astype(f32)
    fr = np.float64(freqs32)
    two_pi = 2 * np.pi

    def red(a):
        m = np.mod(a, two_pi)
        m = np.where(m >= np.pi, m - two_pi, m)
        return m.astype(f32)

    a_sin = red(fr).reshape(BS, 64).T                      # [64, BS]
    a_cos = red(fr + np.pi / 2).reshape(BS, 64).T
    # duplicate across both partition halves -> [128, BS]
    a_sin2 = np.ascontiguousarray(np.concatenate([a_sin, a_sin], 0))
    a_cos2 = np.ascontiguousarray(np.concatenate([a_cos, a_cos], 0))

    wq = np.asarray(q_norm_w, f32).reshape(D, 1)
    wk = np.asarray(k_norm_w, f32).reshape(D, 1)
    wqB = np.roll(wq, 64, axis=0).copy()   # partner-half weight
    wkB = np.roll(wk, 64, axis=0).copy()

    masks_h = np.zeros((1, 128 * B), f32)
    for b in range(B):
        t = lens[b] - (pads[b] // 128 - 1) * 128 if pads[b] > 0 else 128
        masks_h[0, b * 128 + t:(b + 1) * 128] = 1.0

    Wq_ = np.asarray(Wq, f32)
    Wk_ = np.asarray(Wk, f32)
    Wv_ = np.asarray(Wv, f32)
    Wo_ = np.asarray(Wo, f32)

    in_maps = []
    for c in range(N_CORES):
        wqT = np.ascontiguousarray(Wq_[c * EC:(c + 1) * EC, :].T)     # [HID, 512]
        wkvT = np.ascontiguousarray(
            np.concatenate([Wk_[c * D:(c + 1) * D, :],
                            Wv_[c * D:(c + 1) * D, :]], 0).T)         # [HID, 256]
        woT = np.ascontiguousarray(Wo_[:, c * EC:(c + 1) * EC].T)     # [512, HID]
        kT = np.zeros((128, total), f32)
        vC = np.zeros((total, 128), f32)
        for b in range(B):
            nb, ob = lens[b], offs[b]
            if nb > 0:
                kT[:, ob:ob + nb] = kg[b, :nb, c, :].T
                vC[ob:ob + nb, :] = vg[b, :nb, c, :]
        vP = np.ascontiguousarray(
            vC.reshape(total // 128, 128, 128).transpose(1, 0, 2)
            .reshape(128, total))
        in_maps.append(dict(
            xT=xT, wqT=wqT, wkvT=wkvT, woT=woT,
            kT=np.ascontiguousarray(kT), vC=vP,
            a_sin=a_sin2, a_cos=a_cos2,
            wqA=wq, wqB=wqB, wkA=wk, wkB=wkB,
            epsq=np.full((1, 1), D * EPS, f32),
            epsk=np.full((1, 1), EPS, f32),
            masks=masks_h, negrow=np.full((1, 512), -1e30, f32),
            ones=np.ones((128, 1), f32), onesr=np.ones((1, 128), f32),
            ident=np.eye(128, dtype=f32),
        ))
    return in_maps, lens, pads, offs, total


def _build_nc(lens, pads, offs, total, reps=1):
    import concourse.mybir as mybir
    import concourse.tile as tile
    from concourse import bacc

    F32 = mybir.dt.float32
    F32R = mybir.dt.float32r
    AF = mybir.ActivationFunctionType
    OP = mybir.AluOpType

    nc = bacc.Bacc("TRN2", target_bir_lowering=False, debug=False,
                   num_devices=N_CORES)

    xT_d = nc.dram_tensor("xT", [HID, BS], F32R, kind="ExternalInput")
    wqT_d = nc.dram_tensor("wqT", [HID, EC], F32R, kind="ExternalInput")
    wkvT_d = nc.dram_tensor("wkvT", [HID, 2 * D], F32R, kind="ExternalInput")
    woT_d = nc.dram_tensor("woT", [EC, HID], F32R, kind="ExternalInput")
    kT_d = nc.dram_tensor("kT", [128, total], F32R, kind="ExternalInput")
    vC_d = nc.dram_tensor("vC", [128, total], F32R, kind="ExternalInput")
    asin_d = nc.dram_tensor("a_sin", [128, BS], F32, kind="ExternalInput")
    acos_d = nc.dram_tensor("a_cos", [128, BS], F32, kind="ExternalInput")
    wqA_d = nc.dram_tensor("wqA", [128, 1], F32, kind="ExternalInput")
    wqB_d = nc.dram_tensor("wqB", [128, 1], F32, kind="ExternalInput")
    wkA_d = nc.dram_tensor("wkA", [128, 1], F32, kind="ExternalInput")
    wkB_d = nc.dram_tensor("wkB", [128, 1], F32, kind="ExternalInput")
    epsq_d = nc.dram_tensor("epsq", [1, 1], F32, kind="ExternalInput")
    epsk_d = nc.dram_tensor("epsk", [1, 1], F32, kind="ExternalInput")
    masks_d = nc.dram_tensor("masks", [1, 128 * B], F32R, kind="ExternalInput")
    negr_d = nc.dram_tensor("negrow", [1, 512], F32R, kind="ExternalInput")
    ones_d = nc.dram_tensor("ones", [128, 1], F32R, kind="ExternalInput")
    onesr_d = nc.dram_tensor("onesr", [1, 128], F32R, kind="ExternalInput")
    id_d = nc.dram_tensor("ident", [128, 128], F32, kind="ExternalInput")
    out_d = nc.dram_tensor("out", [BS, HID], F32, kind="ExternalOutput")

    # DRAM big-views for batched loads: row-chunk c, partition p, col e.
    xT_v = xT_d.rearrange("(c p) e -> p c e", p=128)       # [128, 32, 512]
    wqT_v = wqT_d.rearrange("(c p) e -> p c e", p=128)     # [128, 32, 512]
    wkvT_v = wkvT_d.rearrange("(c p) e -> p c e", p=128)   # [128, 32, 256]
    woT_v = woT_d.rearrange("(c p) e -> p c e", p=128)     # [128, 4, 4096]

    with tile.TileContext(nc) as tc:
        with tc.tile_pool(name="const", bufs=1) as cpool, \
             tc.tile_pool(name="xpool", bufs=3) as xpool, \
             tc.tile_pool(name="wpool", bufs=3) as wpool, \
             tc.tile_pool(name="persist", bufs=1) as ppool, \
             tc.tile_pool(name="work", bufs=2) as wk, \
             tc.tile_pool(name="probp", bufs=4) as probp, \
             tc.tile_pool(name="ktp", bufs=2) as ktp, \
             tc.tile_pool(name="vp", bufs=3) as vp, \
             tc.tile_pool(name="wop", bufs=8) as wop, \
             tc.tile_pool(name="acc", bufs=6, space="PSUM") as acc, \
             tc.tile_pool(name="rot", bufs=2, space="PSUM") as rot:

            ones = cpool.tile([128, 1], F32R, tag="ones")
            nc.gpsimd.dma_start(out=ones[:, :], in_=ones_d[:, :])
            onesr = cpool.tile([1, 128], F32R, tag="onesr")
            nc.gpsimd.dma_start(out=onesr[:, :], in_=onesr_d[:, :])
            ident = cpool.tile([128, 128], F32, tag="ident")
            nc.gpsimd.dma_start(out=ident[:, :], in_=id_d[:, :])
            wqA = cpool.tile([128, 1], F32, tag="wqA")
            nc.gpsimd.dma_start(out=wqA[:, :], in_=wqA_d[:, :])
            wqBt = cpool.tile([128, 1], F32, tag="wqB")
            nc.gpsimd.dma_start(out=wqBt[:, :], in_=wqB_d[:, :])
            wkA = cpool.tile([128, 1], F32, tag="wkA")
            nc.gpsimd.dma_start(out=wkA[:, :], in_=wkA_d[:, :])
            wkBt = cpool.tile([128, 1], F32, tag="wkB")
            nc.gpsimd.dma_start(out=wkBt[:, :], in_=wkB_d[:, :])
            epsq = cpool.tile([1, 1], F32, tag="epsq")
            nc.gpsimd.dma_start(out=epsq[:, :], in_=epsq_d[:, :])
            epsk = cpool.tile([1, 1], F32, tag="epsk")
            nc.gpsimd.dma_start(out=epsk[:, :], in_=epsk_d[:, :])
            masks = cpool.tile([1, 128 * B], F32R, tag="masks")
            nc.gpsimd.dma_start(out=masks[:, :], in_=masks_d[:, :])
            negrow = cpool.tile([1, 512], F32R, tag="negrow")
            nc.gpsimd.dma_start(out=negrow[:, :], in_=negr_d[:, :])

            def rmsnorm_bcast(ps_in, sq_scale, bias_ap):
                """sumsq over partitions of ps_in -> 1/sqrt(scale*ss + bias)
                broadcast to [128, BS] in a rot-pool psum tile."""
                sq = wk.tile([128, BS], F32R, tag="sq")
                nc.scalar.activation(sq[:, :], ps_in[:, :], AF.Square)
                ps_ss = rot.tile([1, BS], F32, tag="rot")
                nc.tensor.matmul(ps_ss[:, :], ones[:, :], sq[:, :],
                                 start=True, stop=True)
                sqr = wk.tile([1, BS], F32, tag="sqr")
                nc.scalar.activation(sqr[:, :], ps_ss[:, :], AF.Sqrt,
                                     scale=sq_scale, bias=bias_ap[:, :])
                rstd = wk.tile([1, BS], F32R, tag="rstd")
                with nc.allow_low_precision(reason="f32r rounding"):
                    nc.vector.reciprocal(rstd[:, :], sqr[:, :])
                ps_bc = rot.tile([128, BS], F32, tag="rot")
                nc.tensor.matmul(ps_bc[:, :], onesr[:, :], rstd[:, :],
                                 start=True, stop=True)
                return ps_bc

            def rope_scale(ps_in, wA, wB, sin_t, cos_t, ps_bc, dst):
                """NeoX rope + per-token scale: dst[p, :] =
                (ps_in[p]*w[p]*cos[p] -/+ ps_in[partner]*w[partner]*sin[p])
                * bc[p, :]. dst is an f32r SBUF AP [128, BS]."""
                t_sb = wk.tile([128, BS], F32, tag="t_sb")
                nc.scalar.activation(t_sb[:, :], ps_in[:, :], AF.Copy)
                tswap = wk.tile([128, BS], F32, tag="tswap")
                nc.gpsimd.dma_start(out=tswap[0:64, :], in_=t_sb[64:128, :])
                nc.gpsimd.dma_start(out=tswap[64:128, :], in_=t_sb[0:64, :])
                m1 = wk.tile([128, BS], F32, tag="m1")
                nc.vector.scalar_tensor_tensor(
                    m1[:, :], t_sb[:, :], wA[:, :], cos_t[:, :],
                    op0=OP.mult, op1=OP.mult)
                m2 = wk.tile([128, BS], F32, tag="m2")
                nc.vector.scalar_tensor_tensor(
                    m2[:, :], tswap[:, :], wB[:, :], sin_t[:, :],
                    op0=OP.mult, op1=OP.mult)
                rt = wk.tile([128, BS], F32, tag="rt")
                nc.vector.tensor_sub(rt[0:64, :], m1[0:64, :], m2[0:64, :])
                nc.vector.tensor_add(rt[64:128, :], m1[64:128, :], m2[64:128, :])
                nc.vector.tensor_mul(dst, rt[:, :], ps_bc[:, :])

            def body(_it):
                # ---- rope tables ----
                asin = wk.tile([128, BS], F32, tag="asin")
                nc.sync.dma_start(out=asin[:, :], in_=asin_d[:, :])
                acos = wk.tile([128, BS], F32, tag="acos")
                nc.sync.dma_start(out=acos[:, :], in_=acos_d[:, :])
                sin_t = ppool.tile([128, BS], F32, tag="sin")
                nc.scalar.activation(sin_t[:, :], asin[:, :], AF.Sin)
                cos_t = ppool.tile([128, BS], F32, tag="cos")
                nc.scalar.activation(cos_t[:, :], acos[:, :], AF.Sin)

                # ---- phase A: Q,K,V projections in one streamed pass ----
                ps_k = acc.tile([128, BS], F32, tag="acc")
                ps_v = acc.tile([128, BS], F32, tag="acc")
                ps_qs = [acc.tile([128, BS], F32, tag="acc", name=f"ps_q{h}")
                         for h in range(HQC)]
                GRP = 4  # d-chunks per DMA group
                for g in range(NDCH // GRP):
                    xtile = xpool.tile([128, GRP * BS], F32R, tag="xt")
                    nc.sync.dma_start(out=xtile[:, :],
                                      in_=xT_v[:, g * GRP:(g + 1) * GRP, :])
                    wq = wpool.tile([128, GRP * EC], F32R, tag="wq")
                    nc.scalar.dma_start(out=wq[:, :],
                                      in_=wqT_v[:, g * GRP:(g + 1) * GRP, :])
                    wkv = wpool.tile([128, GRP * 256], F32R, tag="wkv")
                    nc.gpsimd.dma_start(out=wkv[:, :],
                                      in_=wkvT_v[:, g * GRP:(g + 1) * GRP, :])
                    for j in range(GRP):
                        dch = g * GRP + j
                        st = dch == 0
                        sp = dch == NDCH - 1
                        xa = xtile[:, j * BS:(j + 1) * BS]
                        nc.tensor.matmul(ps_k[:, :], wkv[:, j * 256:j * 256 + D],
                                         xa, start=st, stop=sp)
                        nc.tensor.matmul(ps_v[:, :],
                                         wkv[:, j * 256 + D:(j + 1) * 256],
                                         xa, start=st, stop=sp)
                        for h in range(HQC):
                            nc.tensor.matmul(
                                ps_qs[h][:, :],
                                wq[:, j * EC + h * D:j * EC + (h + 1) * D],
                                xa, start=st, stop=sp)

                # ---- k: rmsnorm + rope -> k_sb [D, BS] f32r ----
                ps_bc_k = rmsnorm_bcast(ps_k, 1.0 / D, epsk)
                k_sb = ppool.tile([128, BS], F32R, tag="k_sb")
                rope_scale(ps_k, wkA, wkBt, sin_t, cos_t, ps_bc_k, k_sb[:, :])

                # ---- v: transpose to [token, D] per batch -> vt_sb ----
                v_tmp = ppool.tile([128, BS], F32, tag="v_tmp")
                nc.scalar.activation(v_tmp[:, :], ps_v[:, :], AF.Copy)
                vt_sb = ppool.tile([128, BS], F32R, tag="vt_sb")
                for b in range(B):
                    ps_vt = rot.tile([128, 128], F32, tag="rot")
                    nc.tensor.transpose(ps_vt[:, :],
                                        v_tmp[:, b * S:(b + 1) * S],
                                        ident[:, :])
                    nc.vector.tensor_copy(vt_sb[:, b * S:(b + 1) * S],
                                          ps_vt[:, :])

                # ---- q: norm + rope per head ----
                q_sb = ppool.tile([128, HQC * BS], F32R, tag="q_sb")
                for h in range(HQC):
                    # fold the attention scale into the rmsnorm scale:
                    # rstd*SCALE = 1/sqrt(ss + D*eps)
                    ps_bc_q = rmsnorm_bcast(ps_qs[h], 1.0, epsq)
                    rope_scale(ps_qs[h], wqA, wqBt, sin_t, cos_t, ps_bc_q,
                               q_sb[:, h * BS:(h + 1) * BS])

                # q viewed as [128, h, b, s] for per-batch 3D moving operands
                q4 = q_sb.rearrange("p (h b s) -> p h b s", h=HQC, b=B)

                # ---- attention per batch ----
                o_sb = ppool.tile([128, B * 512], F32R, tag="o_sb")
                for b in range(B):
                    ncache = pads[b] // 128
                    nch = ncache + 1
                    tail = lens[b] - (ncache - 1) * 128 if ncache > 0 else 0
                    ps_o = acc.tile([128, 512], F32, tag="acc")
                    ps_sum = acc.tile([1, 512], F32, tag="acc")
                    pending = []

                    def flush(ci_, prob_):
                        nc.tensor.matmul(ps_o[:, :], vtile[ci_], prob_[:, :],
                                         start=(ci_ == 0), stop=(ci_ == nch - 1))
                        nc.tensor.matmul(ps_sum[:, :], ones[:, :], prob_[:, :],
                                         start=(ci_ == 0), stop=(ci_ == nch - 1))

                    vtile = {}
                    # batched K and V loads: groups of 8 chunks
                    kts = []
                    vts = []
                    for g in range((ncache + 7) // 8):
                        c0 = g * 8
                        c1 = min(ncache, c0 + 8)
                        kt = ktp.tile([128, 1024], F32R, tag="kt")
                        nc.sync.dma_start(
                            out=kt[:, :(c1 - c0) * 128],
                            in_=kT_d[:, offs[b] + c0 * 128:offs[b] + c1 * 128])
                        kts.append(kt)
                        vt = vp.tile([128, 1024], F32R, tag="v")
                        nc.gpsimd.dma_start(
                            out=vt[:, :(c1 - c0) * 128],
                            in_=vC_d[:, offs[b] + c0 * 128:offs[b] + c1 * 128])
                        vts.append(vt)
                    for ci in range(nch):
                        is_new = ci == ncache
                        if is_new:
                            k_chunk = k_sb[:, b * S:(b + 1) * S]
                            vtile[ci] = vt_sb[:, b * S:(b + 1) * S]
                        else:
                            k_chunk = kts[ci // 8][:, (ci % 8) * 128:
                                                   (ci % 8 + 1) * 128]
                            vtile[ci] = vts[ci // 8][:, (ci % 8) * 128:
                                                     (ci % 8 + 1) * 128]
                        ps_s = rot.tile([128, 512], F32, tag="rot")
                        masked = ((not is_new) and ci == ncache - 1
                                  and tail < 128)
                        nc.tensor.matmul(ps_s[:, :], k_chunk,
                                         q4[:, :, b, :], start=True,
                                         stop=not masked)
                        if masked:
                            nc.tensor.matmul(
                                ps_s[:, :], masks[:, b * 128:(b + 1) * 128],
                                negrow[:, :], start=False, stop=True)
                        prob = probp.tile([128, 512], F32R, tag="prob")
                        nc.scalar.activation(prob[:, :], ps_s[:, :], AF.Exp)
                        pending.append((ci, prob))
                        if len(pending) > 1:
                            flush(*pending.pop(0))
                    while pending:
                        flush(*pending.pop(0))

                    # normalize: o * (1/sum) broadcast
                    rec = wk.tile([1, 512], F32R, tag="rec")
                    with nc.allow_low_precision(reason="f32r rounding"):
                        nc.vector.reciprocal(rec[:, :], ps_sum[:, :])
                    ps_bc2 = rot.tile([128, 512], F32, tag="rot")
                    nc.tensor.matmul(ps_bc2[:, :], onesr[:, :], rec[:, :],
                                     start=True, stop=True)
                    bc_sb = wk.tile([128, 512], F32, tag="bc_sb")
                    nc.scalar.activation(bc_sb[:, :], ps_bc2[:, :], AF.Copy)
                    nc.vector.tensor_mul(o_sb[:, b * 512:(b + 1) * 512],
                                         ps_o[:, :], bc_sb[:, :])

                # ---- output projection: partial = o @ WoT_c ----
                for quarter in range(4):
                    wos = []
                    for h in range(HQC):
                        wo = wop.tile([128, 1024], F32R, tag="wo",
                                      name=f"wo{quarter}_{h}")
                        nc.sync.dma_start(
                            out=wo[:, :],
                            in_=woT_v[:, h, quarter * 1024:(quarter + 1) * 1024])
                        wos.append(wo)
                    for b in range(B):
                        od = wk.tile([128, 1024], F32, tag="od")
                        for hc in range(2):
                            ps_out = acc.tile([128, 512], F32, tag="acc")
                            for h in range(HQC):
                                nc.tensor.matmul(
                                    ps_out[:, :],
                                    o_sb[:, b * 512 + h * D:
                                         b * 512 + (h + 1) * D],
                                    wos[h][:, hc * 512:(hc + 1) * 512],
                                    start=(h == 0), stop=(h == HQC - 1))
                            if (quarter * B + b) % 2 == 0:
                                nc.vector.tensor_copy(
                                    od[:, hc * 512:(hc + 1) * 512], ps_out[:, :])
                            else:
                                nc.scalar.activation(
                                    od[:, hc * 512:(hc + 1) * 512], ps_out[:, :],
                                    AF.Copy)
                        nc.gpsimd.dma_start(
                            out=out_d[b * S:(b + 1) * S,
                                      quarter * 1024:(quarter + 1) * 1024],
                            in_=od[:, :])

            if reps == 1:
                body(0)
            else:
                with tc.For_i(0, reps, 1) as it:
                    body(it)

    nc.compile()
    return nc


def _get_nc(lens, pads, offs, total, reps=1):
    key = (tuple(lens), total, reps)
    if key not in _CACHE:
        _CACHE[key] = _build_nc(lens, pads, offs, total, reps)
    return _CACHE[key]


def kernel(x, Wq, Wk, Wv, Wo, q_norm_w, k_norm_w, k_cache, v_cache,
           block_table, cache_seqlens):
    from concourse.bass_utils import run_bass_kernel_spmd

    in_maps, lens, pads, offs, total = _prep_host(
        x, Wq, Wk, Wv, Wo, q_norm_w, k_norm_w, k_cache, v_cache,
        block_table, cache_seqlens)
    nc = _get_nc(lens, pads, offs, total, reps=1)
    res = run_bass_kernel_spmd(nc, in_maps, core_ids=list(range(N_CORES)))
    partials = np.stack([r["out"] for r in res.results], 0)
    out = np.sum(partials, axis=0, dtype=np.float64).astype(np.float32)
    return out.reshape(B, S, HID)
